# revision 29
# baseline (speedup 1.0000x reference)
"""Trainium2 Bass kernel for nn_DeepRMSAFeatureExtractor.

Strategy (8 NeuronCores, SPMD, collective-free, batch-sharded):
  exec time is the MAX per-core first-to-last-instruction span, so launch
  skew between cores is free as long as no core waits on another. Each
  core therefore computes ONLY its 2 of the 16 batch rows end-to-end
  (host concatenates the per-core [128,2] outputs -- pure unshard), while
  the weight stream (W0 fp8, 4.2 MB) is replicated per core because every
  output row needs every W0 element and cross-core traffic would import
  launch skew into the measured span.

  vs the previous replicated-full-batch kernel:
    - batch work (attention tanh/exp/softmax chains, MLP) drops 8x
    - the DMA gate-chain is gone: the profile showed it throttled the
      16 SDMA engines to ~35% busy (~107 GB/s aggregate). Chunks now
      stream back-to-back in FIFO ring order at full rate.
    - ~25 small dma_starts (26.7us of Sync-engine issue time) collapse
      into 3 packed loads (f32 / bf16 / f8 packs built host-side by
      pure relayout+cast).
    - DRAM-bounce broadcasts replaced with PE ones-matmul broadcasts;
      link-feature sums are computed directly as [17,2] on partitions.
  Math identical to before: W0's Hm block collapses via
  x_hm @ W0hm = w @ G with G built on-device from fp8 W0hm; the alpha
  block contracts k-tile j over i with rhs = alpha_all[:, j::128].
"""

import sys

sys.path.insert(0, "/opt/trn_rl_repo")

import numpy as np

import concourse.bass as bass  # noqa: F401  (registers AP machinery)
import concourse.bacc as bacc
import concourse.mybir as mybir
import concourse.tile as tile
from concourse import bass_utils

F32 = mybir.dt.float32
BF16 = mybir.dt.bfloat16
F8 = mybir.dt.float8e4

NCORES = 8
BATCH = 16
BPC = BATCH // NCORES   # batches per core = 2
N = 128                 # nodes == HID
HID = 128
M_EDGES = 256
EF = 17
KP = 5

# offsets into the flat `inputs` row (length 86721)
OFF_SD = 0
OFF_SLOT = 256
OFF_SPEC = 261
OFF_LF = 321
OFF_BET = 4673
OFF_ADJ = 4801

MISC_ROWS = 321                  # source_dest 256 + slots 5 + c_band 30 + l_band 30
HM_OFF = 0
MISC_OFF = N * HID               # 16384
ALPHA_OFF = MISC_OFF + MISC_ROWS  # 16705

# w0big chunking (in 128-col k-tiles): w0hm = tiles 0..127, w0a = tiles 128..255.
# The trailing w0a chunks shrink so the final data->alpha-matmul->MLP->out
# dependency chain closes as early as possible.
CHUNK_TILES = [32, 32, 32, 32, 32, 32, 24, 24, 12, 4]

# ---- f32 pack column map ----
FC_BETT = 0    # [128, 2]   bet^T for this core's batches
FC_WHT = 2     # [128, 1]   WH^T
FC_A12 = 3     # [128, 2]   [a1 a2]
FC_A3 = 5      # [128, 1]   a3
FC_WET = 6     # [128, 17]  WE^T
FC_B0 = 23     # [128, 1]
FC_BRT = 24    # [128, 4]   br^T
FC_WE0 = 28    # [128, 1]   WE[:,0] padded to 128 rows
NF32 = 29

# ---- bf16 pack column map ----
BC_WHCOL = 0    # [128, 1]
BC_XMT = 1      # [128, 6]    misc features: [p, t*2+r]
BC_W0M = 7      # [128, 384]  W0 misc block: [p, t*128+h]
BC_IDENT = 391  # [128, 128]  identity
BC_LFT = 519    # [128, 68]   link features: [p, h*34 + r*17 + e]
BC_BETBC = 587  # [128, 256]  bet broadcast: [i, r*128+j]
BC_WR = 843     # [128, 512]  Wr[t] stacked: [p, t*128+h]
BC_ADJ = 1355   # [128, 128]  adj as raw fp8 bytes (bitcast on device)
NBF16 = 1483

NF8 = BPC * N   # adj section [i, r*128+j] (sign only)


def shard_inputs(inputs, WH, WE, a_attn, W0, b0, Wr, br):
    """Host-side prep: slicing / transposition / padding / dtype cast only."""
    f = np.float32
    bf = mybir.dt.np(BF16)
    f8 = mybir.dt.np(F8)
    X = np.asarray(inputs, dtype=f)
    WH = np.asarray(WH, dtype=f).reshape(1, HID)
    WE = np.asarray(WE, dtype=f)
    a_attn = np.asarray(a_attn, dtype=f)
    W0 = np.asarray(W0, dtype=f)
    b0 = np.asarray(b0, dtype=f)
    Wr = np.asarray(Wr, dtype=f)
    br = np.asarray(br, dtype=f)

    # ---- shared (weight) sections ----
    # w0hm8[f, i*128 + h] = W0[i*128 + f, h]
    w0hm8 = (
        W0[HM_OFF:HM_OFF + N * HID].reshape(N, HID, HID)
        .transpose(1, 0, 2).reshape(128, N * HID)
    )
    # w0a8[i, j*128 + h] = W0[ALPHA_OFF + i*128 + j, h]
    w0a8 = W0[ALPHA_OFF:ALPHA_OFF + N * N].reshape(128, N * HID)
    w0big = np.ascontiguousarray(
        np.concatenate([w0hm8, w0a8], axis=1)).astype(f8)  # [128, 32768]

    w0m = np.zeros((3 * 128, HID), dtype=f)
    w0m[:MISC_ROWS] = W0[MISC_OFF:MISC_OFF + MISC_ROWS]
    w0m_dev = w0m.reshape(3, 128, HID).transpose(1, 0, 2).reshape(128, 3 * HID)

    f32_shared = np.zeros((128, NF32), dtype=f)
    f32_shared[:, FC_WHT] = WH[0]
    f32_shared[:, FC_A12 + 0] = a_attn[:HID, 0]
    f32_shared[:, FC_A12 + 1] = a_attn[HID:2 * HID, 0]
    f32_shared[:, FC_A3] = a_attn[2 * HID:, 0]
    # WE is [17, 128]; wet[h, e] = WE[e, h] -> rows h (128), cols e (17)
    f32_shared[:, FC_WET:FC_WET + EF] = WE.T
    f32_shared[:, FC_B0] = b0
    f32_shared[:, FC_BRT:FC_BRT + 4] = br.T
    f32_shared[:EF, FC_WE0] = WE[:, 0]

    bf16_shared = np.zeros((128, NBF16), dtype=f)
    bf16_shared[:, BC_WHCOL] = WH[0]
    bf16_shared[:, BC_W0M:BC_W0M + 384] = w0m_dev
    bf16_shared[:, BC_IDENT:BC_IDENT + 128] = np.eye(128, dtype=f)
    bf16_shared[:, BC_WR:BC_WR + 512] = Wr.transpose(1, 0, 2).reshape(128, 512)

    # ---- per-core (batch-sharded) sections ----
    in_maps = []
    for c in range(NCORES):
        bsel = slice(c * BPC, (c + 1) * BPC)
        Xb = X[bsel]                                           # [2, 86721]
        bet = Xb[:, OFF_BET:OFF_BET + N]                       # [2, 128]
        adj = Xb[:, OFF_ADJ:OFF_ADJ + N * N].reshape(BPC, N, N)
        lf = Xb[:, OFF_LF:OFF_LF + M_EDGES * EF].reshape(BPC, M_EDGES, EF)
        spec = Xb[:, OFF_SPEC:OFF_SPEC + KP * 12].reshape(BPC, KP, 2, 6)

        xm = np.zeros((BPC, 3 * 128), dtype=f)
        xm[:, 0:256] = Xb[:, OFF_SD:OFF_SD + 256]
        xm[:, 256:261] = Xb[:, OFF_SLOT:OFF_SLOT + KP]
        xm[:, 261:291] = spec[:, :, 0, :].reshape(BPC, 30)
        xm[:, 291:321] = spec[:, :, 1, :].reshape(BPC, 30)

        f32p = f32_shared.copy()
        f32p[:, FC_BETT:FC_BETT + BPC] = bet.T

        bf16p = bf16_shared.copy()
        # xmt[p, t*2 + r] = xm[r, t*128 + p]
        bf16p[:, BC_XMT:BC_XMT + 3 * BPC] = (
            xm.T.reshape(3, 128, BPC).transpose(1, 0, 2).reshape(128, 3 * BPC)
        )
        # lft[p, h*34 + r*17 + e] = lf[r, h*128 + p, e]
        bf16p[:, BC_LFT:BC_LFT + 2 * BPC * EF] = (
            lf.transpose(1, 0, 2).reshape(2, 128, BPC * EF)
            .transpose(1, 0, 2).reshape(128, 2 * BPC * EF)
        )
        # betbc[i, r*128 + j] = bet[r, j]
        bf16p[:, BC_BETBC:BC_BETBC + BPC * N] = np.broadcast_to(
            bet.reshape(1, BPC * N), (128, BPC * N))

        f8p = np.ascontiguousarray(
            adj.transpose(1, 0, 2).reshape(128, BPC * N)).astype(f8)

        bfbytes = np.ascontiguousarray(bf16p.astype(bf)).view(np.uint8)
        bfbytes[:, 2 * BC_ADJ:2 * BC_ADJ + NF8] = f8p.view(np.uint8)

        in_maps.append({
            "f32p": np.ascontiguousarray(f32p),
            "bf16p": bfbytes.view(bf),
            "br3": np.ascontiguousarray(br[3:4, :]),
            "w0big": w0big,
        })
    return in_maps


def build_nc():
    nc = bacc.Bacc("TRN2", target_bir_lowering=False, debug=False,
                   num_devices=NCORES)
    AF = mybir.ActivationFunctionType
    OP = mybir.AluOpType

    t_f32p = nc.dram_tensor("f32p", [128, NF32], F32, kind="ExternalInput").ap()
    t_bf16p = nc.dram_tensor("bf16p", [128, NBF16], BF16, kind="ExternalInput").ap()
    t_br3 = nc.dram_tensor("br3", [1, 128], F32, kind="ExternalInput").ap()
    t_w0big = nc.dram_tensor("w0big", [128, 32768], F8, kind="ExternalInput").ap()
    t_out = nc.dram_tensor("out", [BPC, 128], F32, kind="ExternalOutput").ap()

    with tile.TileContext(nc) as tc:
        with tc.tile_pool(name="sb", bufs=1) as sb, \
             tc.tile_pool(name="ps", bufs=1, space="PSUM") as ps:

            # ------------------------------------ DMA issue (single FIFO ring)
            # All data loads ride ONE HWDGE ring (SP) so they drain in
            # program order: packs first, then W0 chunks strictly in
            # consumption order. With fixed aggregate bandwidth, item X
            # completes at (bytes ahead of X)/BW -- a second parallel ring
            # just interleaves and delays early chunks (measured +5us stall
            # on the G-build). Only the tiny out-store uses the ACT ring.
            f32v = sb.tile([128, NF32], F32, tag="f32p")
            nc.sync.dma_start(f32v[:], t_f32p)
            bf16v = sb.tile([128, NBF16], BF16, tag="bf16p")
            nc.sync.dma_start(bf16v[:], t_bf16p)
            br3row = sb.tile([1, 128], F32, tag="br3")
            nc.sync.dma_start(br3row[:], t_br3)
            w0c = []          # per chunk: (tile, start_tile, ntiles)
            tile0 = 0
            for ci, nt in enumerate(CHUNK_TILES):
                t = sb.tile([128, nt * 128], F8, tag=f"w0c{ci}")
                nc.sync.dma_start(t[:], t_w0big[:, tile0 * 128:(tile0 + nt) * 128])
                w0c.append((t, tile0, nt))
                tile0 += nt
            # k-tile index -> (chunk tile, col offset)
            tile_view = {}
            for t, t0, nt in w0c:
                for k in range(nt):
                    tile_view[t0 + k] = (t, k * 128)

            # views into the packs
            bett = f32v[:, FC_BETT:FC_BETT + BPC]
            wht = f32v[:, FC_WHT:FC_WHT + 1]
            a12 = f32v[:, FC_A12:FC_A12 + 2]
            a3 = f32v[:, FC_A3:FC_A3 + 1]
            wet = f32v[:, FC_WET:FC_WET + EF]
            b0v = f32v[:, FC_B0:FC_B0 + 1]
            brT = f32v[:, FC_BRT:FC_BRT + 4]
            we0 = f32v[:, FC_WE0:FC_WE0 + 1]
            wrv = bf16v[:, BC_WR:BC_WR + 512]
            whcol = bf16v[:, BC_WHCOL:BC_WHCOL + 1]
            xmt = bf16v[:, BC_XMT:BC_XMT + 3 * BPC]
            w0m = bf16v[:, BC_W0M:BC_W0M + 384]
            adj_sb = bf16v[:, BC_ADJ:BC_ADJ + NF8 // 2].bitcast(F8)
            ident = bf16v[:, BC_IDENT:BC_IDENT + 128]
            lft = bf16v[:, BC_LFT:BC_LFT + 2 * BPC * EF]
            betbc = bf16v[:, BC_BETBC:BC_BETBC + BPC * N]

            # -------------------------------------------- constants (no DMA)
            onesrow = sb.tile([1, 128], F32, tag="onesrow")
            nc.vector.memset(onesrow[:], 1.0)
            onescol = sb.tile([128, 1], BF16, tag="onescol")
            nc.vector.memset(onescol[:], 1.0)
            neg31 = sb.tile([128, 1], F32, tag="neg31")
            nc.vector.memset(neg31[:], -31.0)

            # -------------------------------------------- tiny weight math
            # [q, k] = WH @ [a1 a2]; broadcast to all partitions via ones-matmul
            ps_qk = ps.tile([1, 2], F32, tag="small")
            nc.tensor.matmul(ps_qk[:], wht, a12, start=True, stop=True)
            qk_sb = sb.tile([1, 2], F32, tag="qksb")
            nc.vector.tensor_copy(qk_sb[:], ps_qk[:])
            ps_qkbc = ps.tile([128, 2], F32, tag="bc")
            nc.tensor.matmul(ps_qkbc[:], onesrow[:], qk_sb[:], start=True, stop=True)
            qkbc = sb.tile([128, 2], F32, tag="qkbc")
            nc.vector.tensor_copy(qkbc[:], ps_qkbc[:])
            q_bc = qkbc[:, 0:1]
            k_bc = qkbc[:, 1:2]

            # a3e[e] = sum_h WE[e,h]*a3[h];  lhsT2 = [a3e, WE[:,0]]  [17,2]
            ps_a3e = ps.tile([17, 1], F32, tag="small")
            nc.tensor.matmul(ps_a3e[:], wet, a3, start=True, stop=True)
            lhsT2 = sb.tile([17, 2], F32, tag="lhsT2")
            nc.vector.tensor_copy(lhsT2[:, 0:1], ps_a3e[:])
            nc.vector.tensor_copy(lhsT2[:, 1:2], we0[0:EF, :])

            # link-feature sums on partitions: lfmT[e, r] = sum_m lf[r, m, e]
            ps_lf = ps.tile([EF, BPC], F32, tag="small")
            for r in range(BPC):
                for h in range(2):
                    nc.tensor.matmul(
                        ps_lf[:, r:r + 1],
                        lft[:, h * 34 + r * EF:h * 34 + (r + 1) * EF],
                        onescol[:], start=(h == 0), stop=(h == 1))
            lfmT = sb.tile([EF, BPC], F32, tag="lfmT")
            nc.vector.tensor_copy(lfmT[:], ps_lf[:])

            # seec_row = [se_0, se_1, ec0_0, ec0_1] / 256 ; broadcast to [128,4]
            ps_seec = ps.tile([1, 2 * BPC], F32, tag="small")
            nc.tensor.matmul(ps_seec[:, 0:BPC], lhsT2[:, 0:1], lfmT[:],
                             start=True, stop=True)
            nc.tensor.matmul(ps_seec[:, BPC:2 * BPC], lhsT2[:, 1:2], lfmT[:],
                             start=True, stop=True)
            seec_row = sb.tile([1, 2 * BPC], F32, tag="seecrow")
            nc.scalar.activation(seec_row[:], ps_seec[:], AF.Copy, bias=0.0,
                                 scale=1.0 / M_EDGES)
            ps_seecbc = ps.tile([128, 2 * BPC], F32, tag="bc")
            nc.tensor.matmul(ps_seecbc[:], onesrow[:], seec_row[:],
                             start=True, stop=True)
            seecbc = sb.tile([128, 2 * BPC], F32, tag="seecbc")
            nc.vector.tensor_copy(seecbc[:], ps_seecbc[:])
            sebc = seecbc[:, 0:BPC]

            # pp[i,r] = q*bet[r,i] + se[r]
            pp = sb.tile([128, BPC], F32, tag="pp")
            nc.vector.scalar_tensor_tensor(pp[:], bett, q_bc, sebc,
                                           OP.mult, OP.add)

            # -------------------------------------------- attention (BPC tiles)
            alpha_all = sb.tile([128, BPC * N], BF16, tag="alpha")
            wT_sb = sb.tile([128, BPC], BF16, tag="wT")
            for r in range(BPC):
                bsl = slice(r * 128, (r + 1) * 128)
                tt = sb.tile([128, 128], BF16, tag=f"tt{r}")
                nc.scalar.activation(tt[:], betbc[:, bsl], AF.Tanh,
                                     bias=pp[:, r:r + 1], scale=k_bc)
                m01 = sb.tile([128, 128], BF16, tag=f"m01{r}")
                nc.vector.tensor_scalar(m01[:], adj_sb[:, bsl], 0.0, None,
                                        OP.is_gt)
                stt = sb.tile([128, 128], BF16, tag=f"stt{r}")
                nc.vector.scalar_tensor_tensor(stt[:], m01[:], 31.0, tt[:],
                                               OP.mult, OP.add)
                un = sb.tile([128, 128], BF16, tag=f"un{r}")
                rowsum = sb.tile([128, 1], F32, tag=f"rows{r}")
                nc.scalar.activation(un[:], stt[:], AF.Exp,
                                     bias=neg31[:], scale=1.0,
                                     accum_out=rowsum[:])
                recip = sb.tile([128, 1], F32, tag=f"recip{r}")
                nc.vector.reciprocal(recip[:], rowsum[:])
                nc.vector.tensor_scalar(alpha_all[:, bsl], un[:], recip[:],
                                        None, OP.mult)
                tmp = sb.tile([128, 128], BF16, tag=f"wtmp{r}")
                nc.gpsimd.tensor_tensor(tmp[:], un[:], betbc[:, bsl], OP.mult)
                r_un = sb.tile([128, 1], F32, tag=f"run{r}")
                nc.vector.reduce_sum(r_un[:], tmp[:], axis=mybir.AxisListType.X)
                nc.gpsimd.tensor_scalar(wT_sb[:, r:r + 1], r_un[:], recip[:],
                                        seecbc[:, BPC + r:BPC + r + 1],
                                        OP.mult, OP.mult)

            # -------------------------------------------- misc block into main
            ps_main = ps.tile([128, BPC], F32, tag="main")
            for t in range(3):
                nc.tensor.matmul(ps_main[:],
                                 w0m[:, t * 128:(t + 1) * 128],
                                 xmt[:, t * BPC:(t + 1) * BPC],
                                 start=(t == 0), stop=False)

            # output bias row early: ps_out accumulates br3 now, x4 @ Wr3 later
            ps_out = ps.tile([BPC, 128], F32, tag="psout")
            nc.tensor.matmul(ps_out[:], onesrow[0:1, 0:BPC], br3row[:],
                             start=True, stop=False)

            # -------------------------------------------- G^T build (128 MMs)
            ps_gt = ps.tile([128, 128], F32, tag="gt")
            for i in range(128):
                ch, off = tile_view[i]
                nc.tensor.matmul(ps_gt[:, i:i + 1], ch[:, off:off + 128],
                                 whcol, start=True, stop=True)
            gt_sb = sb.tile([128, 128], BF16, tag="gtsb")
            nc.vector.tensor_copy(gt_sb[:], ps_gt[:])
            ps_g = ps.tile([128, 128], BF16, tag="g")
            nc.tensor.transpose(ps_g[:], gt_sb[:], ident)
            g_sb = sb.tile([128, 128], BF16, tag="gsb")
            nc.vector.tensor_copy(g_sb[:], ps_g[:])

            # -------------------------------------------- Hm block: w @ G
            nc.tensor.matmul(ps_main[:], g_sb[:], wT_sb[:],
                             start=False, stop=False)

            # -------------------------------------------- alpha block (128 MMs)
            for j in range(128):
                ch, off = tile_view[128 + j]
                nc.tensor.matmul(ps_main[:], ch[:, off:off + 128],
                                 alpha_all[:, j:BPC * N:128],
                                 start=False, stop=(j == 127))

            # -------------------------------------------- bias+relu + MLP
            # layers 0-2 transposed [128h, b]; layer 3 emits [b, 128h] directly
            # (lhsT = x4^T, bias pre-accumulated via ones-matmul) so the out
            # DMA is 2 contiguous rows. relu+bias fused on DVE (one op/hop).
            xT = sb.tile([128, BPC], BF16, tag="x1T")
            nc.vector.tensor_scalar(xT[:], ps_main[:], b0v, 0.0,
                                    OP.add, OP.max)
            for t in range(3):
                ps_l = ps.tile([128, BPC], F32, tag="psl")
                nc.tensor.matmul(ps_l[:], wrv[:, t * 128:(t + 1) * 128],
                                 xT[:], start=True, stop=True)
                xT_next = sb.tile([128, BPC], BF16, tag=f"x{t + 2}T")
                nc.vector.tensor_scalar(xT_next[:], ps_l[:], brT[:, t:t + 1],
                                        0.0, OP.add, OP.max)
                xT = xT_next
            nc.tensor.matmul(ps_out[:], xT[:], wrv[:, 384:512],
                             start=False, stop=True)
            out_sb = sb.tile([BPC, 128], F32, tag="outsb")
            nc.vector.tensor_scalar(out_sb[:], ps_out[:], 0.0, None, OP.max)
            nc.sync.dma_start(t_out, out_sb[:])
    nc.compile()
    return nc


_compiled_nc = None


def get_nc():
    global _compiled_nc
    if _compiled_nc is None:
        _compiled_nc = build_nc()
    return _compiled_nc


def gather(results):
    """[2, 128] per core -> [16, 128] full output (pure unshard)."""
    return np.concatenate(
        [np.asarray(results[c]["out"], dtype=np.float32)
         for c in range(NCORES)], axis=0)


def kernel(**inputs):
    nc = get_nc()
    in_maps = shard_inputs(**inputs)
    res = bass_utils.run_bass_kernel_spmd(nc, in_maps, core_ids=list(range(NCORES)))
    return gather(res.results)


if __name__ == "__main__":
    nc = build_nc()
    print("build + compile OK;", len(nc.main_func.blocks), "blocks")


# revision 30
# speedup vs baseline: 1.0452x; 1.0452x over previous
"""Trainium2 Bass kernel for nn_DeepRMSAFeatureExtractor.

Strategy (8 NeuronCores, SPMD, collective-free, batch-sharded):
  exec time is the MAX per-core first-to-last-instruction span, so launch
  skew between cores is free as long as no core waits on another. Each
  core therefore computes ONLY its 2 of the 16 batch rows end-to-end
  (host concatenates the per-core [128,2] outputs -- pure unshard), while
  the weight stream (W0 fp8, 4.2 MB) is replicated per core because every
  output row needs every W0 element and cross-core traffic would import
  launch skew into the measured span.

  vs the previous replicated-full-batch kernel:
    - batch work (attention tanh/exp/softmax chains, MLP) drops 8x
    - the DMA gate-chain is gone: the profile showed it throttled the
      16 SDMA engines to ~35% busy (~107 GB/s aggregate). Chunks now
      stream back-to-back in FIFO ring order at full rate.
    - ~25 small dma_starts (26.7us of Sync-engine issue time) collapse
      into 2 packed loads (an f32 pack and a bf16 pack that carries the
      fp8 adjacency section via an on-device bitcast view; host-side
      prep is pure relayout+cast) plus a tiny [1,128] br3 row.
    - DRAM-bounce broadcasts replaced with PE ones-matmul broadcasts;
      link-feature sums are computed directly as [17,2] on partitions.
  Math identical to before: W0's Hm block collapses via
  x_hm @ W0hm = w @ G with G built on-device from fp8 W0hm; the alpha
  block contracts k-tile j over i with rhs = alpha_all[:, j::128].
  Wr + MLP activations run in bf16 with DVE-fused bias+relu hops; the
  final layer emits [2,128] directly (bias pre-accumulated in PSUM) so
  the out-store is 2 contiguous descriptors.

  Measured floor notes (do not re-litigate without new evidence):
  - trivial passthrough kernel = 13.3us: prologue/epilogue are fixed
    runtime cost, independent of program size.
  - single-core run == 8-core mean: no HBM contention; ~280 GB/s avg
    is the per-core rate for this pattern.
  - drain order must match consumption order on ONE ring: a second
    parallel ring, SWDGE packs, or 1MB chunks all measured slower.
  - DMA-transpose is serialized against the stream by Tile (+6us).
"""

import sys

sys.path.insert(0, "/opt/trn_rl_repo")

import numpy as np

import concourse.bass as bass  # noqa: F401  (registers AP machinery)
import concourse.bacc as bacc
import concourse.mybir as mybir
import concourse.tile as tile
from concourse import bass_utils

F32 = mybir.dt.float32
BF16 = mybir.dt.bfloat16
F8 = mybir.dt.float8e4

NCORES = 8
BATCH = 16
BPC = BATCH // NCORES   # batches per core = 2
N = 128                 # nodes == HID
HID = 128
M_EDGES = 256
EF = 17
KP = 5

# offsets into the flat `inputs` row (length 86721)
OFF_SD = 0
OFF_SLOT = 256
OFF_SPEC = 261
OFF_LF = 321
OFF_BET = 4673
OFF_ADJ = 4801

MISC_ROWS = 321                  # source_dest 256 + slots 5 + c_band 30 + l_band 30
HM_OFF = 0
MISC_OFF = N * HID               # 16384
ALPHA_OFF = MISC_OFF + MISC_ROWS  # 16705

# w0big chunking (in 128-col k-tiles): w0hm = tiles 0..127, w0a = tiles 128..255.
# The trailing w0a chunks shrink so the final data->alpha-matmul->MLP->out
# dependency chain closes as early as possible.
CHUNK_TILES = [32, 32, 32, 32, 32, 32, 24, 24, 12, 4]

# ---- f32 pack column map ----
FC_BETT = 0    # [128, 2]   bet^T for this core's batches
FC_WHT = 2     # [128, 1]   WH^T
FC_A12 = 3     # [128, 2]   [a1 a2]
FC_A3 = 5      # [128, 1]   a3
FC_WET = 6     # [128, 17]  WE^T
FC_B0 = 23     # [128, 1]
FC_BRT = 24    # [128, 4]   br^T
FC_WE0 = 28    # [128, 1]   WE[:,0] padded to 128 rows
NF32 = 29

# ---- bf16 pack column map ----
BC_WHCOL = 0    # [128, 1]
BC_XMT = 1      # [128, 6]    misc features: [p, t*2+r]
BC_W0M = 7      # [128, 384]  W0 misc block: [p, t*128+h]
BC_IDENT = 391  # [128, 128]  identity
BC_LFT = 519    # [128, 68]   link features: [p, h*34 + r*17 + e]
BC_BETBC = 587  # [128, 256]  bet broadcast: [i, r*128+j]
BC_WR = 843     # [128, 512]  Wr[t] stacked: [p, t*128+h]
BC_ADJ = 1355   # [128, 128]  adj as raw fp8 bytes (bitcast on device)
NBF16 = 1483

NF8 = BPC * N   # adj section [i, r*128+j] (sign only)


def shard_inputs(inputs, WH, WE, a_attn, W0, b0, Wr, br):
    """Host-side prep: slicing / transposition / padding / dtype cast only."""
    f = np.float32
    bf = mybir.dt.np(BF16)
    f8 = mybir.dt.np(F8)
    X = np.asarray(inputs, dtype=f)
    WH = np.asarray(WH, dtype=f).reshape(1, HID)
    WE = np.asarray(WE, dtype=f)
    a_attn = np.asarray(a_attn, dtype=f)
    W0 = np.asarray(W0, dtype=f)
    b0 = np.asarray(b0, dtype=f)
    Wr = np.asarray(Wr, dtype=f)
    br = np.asarray(br, dtype=f)

    # ---- shared (weight) sections ----
    # w0hm8[f, i*128 + h] = W0[i*128 + f, h]
    w0hm8 = (
        W0[HM_OFF:HM_OFF + N * HID].reshape(N, HID, HID)
        .transpose(1, 0, 2).reshape(128, N * HID)
    )
    # w0a8[i, j*128 + h] = W0[ALPHA_OFF + i*128 + j, h]
    w0a8 = W0[ALPHA_OFF:ALPHA_OFF + N * N].reshape(128, N * HID)
    w0big = np.ascontiguousarray(
        np.concatenate([w0hm8, w0a8], axis=1)).astype(f8)  # [128, 32768]

    w0m = np.zeros((3 * 128, HID), dtype=f)
    w0m[:MISC_ROWS] = W0[MISC_OFF:MISC_OFF + MISC_ROWS]
    w0m_dev = w0m.reshape(3, 128, HID).transpose(1, 0, 2).reshape(128, 3 * HID)

    f32_shared = np.zeros((128, NF32), dtype=f)
    f32_shared[:, FC_WHT] = WH[0]
    f32_shared[:, FC_A12 + 0] = a_attn[:HID, 0]
    f32_shared[:, FC_A12 + 1] = a_attn[HID:2 * HID, 0]
    f32_shared[:, FC_A3] = a_attn[2 * HID:, 0]
    # WE is [17, 128]; wet[h, e] = WE[e, h] -> rows h (128), cols e (17)
    f32_shared[:, FC_WET:FC_WET + EF] = WE.T
    f32_shared[:, FC_B0] = b0
    f32_shared[:, FC_BRT:FC_BRT + 4] = br.T
    f32_shared[:EF, FC_WE0] = WE[:, 0]

    bf16_shared = np.zeros((128, NBF16), dtype=f)
    bf16_shared[:, BC_WHCOL] = WH[0]
    bf16_shared[:, BC_W0M:BC_W0M + 384] = w0m_dev
    bf16_shared[:, BC_IDENT:BC_IDENT + 128] = np.eye(128, dtype=f)
    bf16_shared[:, BC_WR:BC_WR + 512] = Wr.transpose(1, 0, 2).reshape(128, 512)

    # ---- per-core (batch-sharded) sections ----
    in_maps = []
    for c in range(NCORES):
        bsel = slice(c * BPC, (c + 1) * BPC)
        Xb = X[bsel]                                           # [2, 86721]
        bet = Xb[:, OFF_BET:OFF_BET + N]                       # [2, 128]
        adj = Xb[:, OFF_ADJ:OFF_ADJ + N * N].reshape(BPC, N, N)
        lf = Xb[:, OFF_LF:OFF_LF + M_EDGES * EF].reshape(BPC, M_EDGES, EF)
        spec = Xb[:, OFF_SPEC:OFF_SPEC + KP * 12].reshape(BPC, KP, 2, 6)

        xm = np.zeros((BPC, 3 * 128), dtype=f)
        xm[:, 0:256] = Xb[:, OFF_SD:OFF_SD + 256]
        xm[:, 256:261] = Xb[:, OFF_SLOT:OFF_SLOT + KP]
        xm[:, 261:291] = spec[:, :, 0, :].reshape(BPC, 30)
        xm[:, 291:321] = spec[:, :, 1, :].reshape(BPC, 30)

        f32p = f32_shared.copy()
        f32p[:, FC_BETT:FC_BETT + BPC] = bet.T

        bf16p = bf16_shared.copy()
        # xmt[p, t*2 + r] = xm[r, t*128 + p]
        bf16p[:, BC_XMT:BC_XMT + 3 * BPC] = (
            xm.T.reshape(3, 128, BPC).transpose(1, 0, 2).reshape(128, 3 * BPC)
        )
        # lft[p, h*34 + r*17 + e] = lf[r, h*128 + p, e]
        bf16p[:, BC_LFT:BC_LFT + 2 * BPC * EF] = (
            lf.transpose(1, 0, 2).reshape(2, 128, BPC * EF)
            .transpose(1, 0, 2).reshape(128, 2 * BPC * EF)
        )
        # betbc[i, r*128 + j] = bet[r, j]
        bf16p[:, BC_BETBC:BC_BETBC + BPC * N] = np.broadcast_to(
            bet.reshape(1, BPC * N), (128, BPC * N))

        f8p = np.ascontiguousarray(
            adj.transpose(1, 0, 2).reshape(128, BPC * N)).astype(f8)

        bfbytes = np.ascontiguousarray(bf16p.astype(bf)).view(np.uint8)
        bfbytes[:, 2 * BC_ADJ:2 * BC_ADJ + NF8] = f8p.view(np.uint8)

        in_maps.append({
            "f32p": np.ascontiguousarray(f32p),
            "bf16p": bfbytes.view(bf),
            "br3": np.ascontiguousarray(br[3:4, :]),
            "w0big": w0big,
        })
    return in_maps


def build_nc():
    nc = bacc.Bacc("TRN2", target_bir_lowering=False, debug=False,
                   num_devices=NCORES)
    AF = mybir.ActivationFunctionType
    OP = mybir.AluOpType

    t_f32p = nc.dram_tensor("f32p", [128, NF32], F32, kind="ExternalInput").ap()
    t_bf16p = nc.dram_tensor("bf16p", [128, NBF16], BF16, kind="ExternalInput").ap()
    t_br3 = nc.dram_tensor("br3", [1, 128], F32, kind="ExternalInput").ap()
    t_w0big = nc.dram_tensor("w0big", [128, 32768], F8, kind="ExternalInput").ap()
    t_out = nc.dram_tensor("out", [BPC, 128], F32, kind="ExternalOutput").ap()

    with tile.TileContext(nc) as tc:
        with tc.tile_pool(name="sb", bufs=1) as sb, \
             tc.tile_pool(name="ps", bufs=1, space="PSUM") as ps:

            # ------------------------------------ DMA issue (single FIFO ring)
            # All data loads ride ONE HWDGE ring (SP) so they drain in
            # program order: packs first, then W0 chunks strictly in
            # consumption order. With fixed aggregate bandwidth, item X
            # completes at (bytes ahead of X)/BW -- a second parallel ring
            # just interleaves and delays early chunks (measured +5us stall
            # on the G-build). Only the tiny out-store uses the ACT ring.
            f32v = sb.tile([128, NF32], F32, tag="f32p")
            nc.sync.dma_start(f32v[:], t_f32p)
            bf16v = sb.tile([128, NBF16], BF16, tag="bf16p")
            nc.sync.dma_start(bf16v[:], t_bf16p)
            br3row = sb.tile([1, 128], F32, tag="br3")
            nc.sync.dma_start(br3row[:], t_br3)
            w0c = []          # per chunk: (tile, start_tile, ntiles)
            tile0 = 0
            for ci, nt in enumerate(CHUNK_TILES):
                t = sb.tile([128, nt * 128], F8, tag=f"w0c{ci}")
                nc.sync.dma_start(t[:], t_w0big[:, tile0 * 128:(tile0 + nt) * 128])
                w0c.append((t, tile0, nt))
                tile0 += nt
            # k-tile index -> (chunk tile, col offset)
            tile_view = {}
            for t, t0, nt in w0c:
                for k in range(nt):
                    tile_view[t0 + k] = (t, k * 128)

            # views into the packs
            bett = f32v[:, FC_BETT:FC_BETT + BPC]
            wht = f32v[:, FC_WHT:FC_WHT + 1]
            a12 = f32v[:, FC_A12:FC_A12 + 2]
            a3 = f32v[:, FC_A3:FC_A3 + 1]
            wet = f32v[:, FC_WET:FC_WET + EF]
            b0v = f32v[:, FC_B0:FC_B0 + 1]
            brT = f32v[:, FC_BRT:FC_BRT + 4]
            we0 = f32v[:, FC_WE0:FC_WE0 + 1]
            wrv = bf16v[:, BC_WR:BC_WR + 512]
            whcol = bf16v[:, BC_WHCOL:BC_WHCOL + 1]
            xmt = bf16v[:, BC_XMT:BC_XMT + 3 * BPC]
            w0m = bf16v[:, BC_W0M:BC_W0M + 384]
            adj_sb = bf16v[:, BC_ADJ:BC_ADJ + NF8 // 2].bitcast(F8)
            ident = bf16v[:, BC_IDENT:BC_IDENT + 128]
            lft = bf16v[:, BC_LFT:BC_LFT + 2 * BPC * EF]
            betbc = bf16v[:, BC_BETBC:BC_BETBC + BPC * N]

            # -------------------------------------------- constants (no DMA)
            onesrow = sb.tile([1, 128], F32, tag="onesrow")
            nc.vector.memset(onesrow[:], 1.0)
            onescol = sb.tile([128, 1], BF16, tag="onescol")
            nc.vector.memset(onescol[:], 1.0)
            neg31 = sb.tile([128, 1], F32, tag="neg31")
            nc.vector.memset(neg31[:], -31.0)

            # -------------------------------------------- tiny weight math
            # [q, k] = WH @ [a1 a2]; broadcast to all partitions via ones-matmul
            ps_qk = ps.tile([1, 2], F32, tag="small")
            nc.tensor.matmul(ps_qk[:], wht, a12, start=True, stop=True)
            qk_sb = sb.tile([1, 2], F32, tag="qksb")
            nc.vector.tensor_copy(qk_sb[:], ps_qk[:])
            ps_qkbc = ps.tile([128, 2], F32, tag="bc")
            nc.tensor.matmul(ps_qkbc[:], onesrow[:], qk_sb[:], start=True, stop=True)
            qkbc = sb.tile([128, 2], F32, tag="qkbc")
            nc.vector.tensor_copy(qkbc[:], ps_qkbc[:])
            q_bc = qkbc[:, 0:1]
            k_bc = qkbc[:, 1:2]

            # a3e[e] = sum_h WE[e,h]*a3[h];  lhsT2 = [a3e, WE[:,0]]  [17,2]
            ps_a3e = ps.tile([17, 1], F32, tag="small")
            nc.tensor.matmul(ps_a3e[:], wet, a3, start=True, stop=True)
            lhsT2 = sb.tile([17, 2], F32, tag="lhsT2")
            nc.vector.tensor_copy(lhsT2[:, 0:1], ps_a3e[:])
            nc.vector.tensor_copy(lhsT2[:, 1:2], we0[0:EF, :])

            # link-feature sums on partitions: lfmT[e, r] = sum_m lf[r, m, e]
            ps_lf = ps.tile([EF, BPC], F32, tag="small")
            for r in range(BPC):
                for h in range(2):
                    nc.tensor.matmul(
                        ps_lf[:, r:r + 1],
                        lft[:, h * 34 + r * EF:h * 34 + (r + 1) * EF],
                        onescol[:], start=(h == 0), stop=(h == 1))
            lfmT = sb.tile([EF, BPC], F32, tag="lfmT")
            nc.vector.tensor_copy(lfmT[:], ps_lf[:])

            # seec_row = [se_0, se_1, ec0_0, ec0_1] / 256 ; broadcast to [128,4]
            ps_seec = ps.tile([1, 2 * BPC], F32, tag="small")
            nc.tensor.matmul(ps_seec[:, 0:BPC], lhsT2[:, 0:1], lfmT[:],
                             start=True, stop=True)
            nc.tensor.matmul(ps_seec[:, BPC:2 * BPC], lhsT2[:, 1:2], lfmT[:],
                             start=True, stop=True)
            seec_row = sb.tile([1, 2 * BPC], F32, tag="seecrow")
            nc.scalar.activation(seec_row[:], ps_seec[:], AF.Copy, bias=0.0,
                                 scale=1.0 / M_EDGES)
            ps_seecbc = ps.tile([128, 2 * BPC], F32, tag="bc")
            nc.tensor.matmul(ps_seecbc[:], onesrow[:], seec_row[:],
                             start=True, stop=True)
            seecbc = sb.tile([128, 2 * BPC], F32, tag="seecbc")
            nc.vector.tensor_copy(seecbc[:], ps_seecbc[:])
            sebc = seecbc[:, 0:BPC]

            # pp[i,r] = q*bet[r,i] + se[r]
            pp = sb.tile([128, BPC], F32, tag="pp")
            nc.vector.scalar_tensor_tensor(pp[:], bett, q_bc, sebc,
                                           OP.mult, OP.add)

            # -------------------------------------------- attention (BPC tiles)
            alpha_all = sb.tile([128, BPC * N], BF16, tag="alpha")
            wT_sb = sb.tile([128, BPC], BF16, tag="wT")
            for r in range(BPC):
                bsl = slice(r * 128, (r + 1) * 128)
                tt = sb.tile([128, 128], BF16, tag=f"tt{r}")
                nc.scalar.activation(tt[:], betbc[:, bsl], AF.Tanh,
                                     bias=pp[:, r:r + 1], scale=k_bc)
                m01 = sb.tile([128, 128], BF16, tag=f"m01{r}")
                nc.vector.tensor_scalar(m01[:], adj_sb[:, bsl], 0.0, None,
                                        OP.is_gt)
                stt = sb.tile([128, 128], BF16, tag=f"stt{r}")
                nc.vector.scalar_tensor_tensor(stt[:], m01[:], 31.0, tt[:],
                                               OP.mult, OP.add)
                un = sb.tile([128, 128], BF16, tag=f"un{r}")
                rowsum = sb.tile([128, 1], F32, tag=f"rows{r}")
                nc.scalar.activation(un[:], stt[:], AF.Exp,
                                     bias=neg31[:], scale=1.0,
                                     accum_out=rowsum[:])
                recip = sb.tile([128, 1], F32, tag=f"recip{r}")
                nc.vector.reciprocal(recip[:], rowsum[:])
                nc.vector.tensor_scalar(alpha_all[:, bsl], un[:], recip[:],
                                        None, OP.mult)
                tmp = sb.tile([128, 128], BF16, tag=f"wtmp{r}")
                nc.gpsimd.tensor_tensor(tmp[:], un[:], betbc[:, bsl], OP.mult)
                r_un = sb.tile([128, 1], F32, tag=f"run{r}")
                nc.vector.reduce_sum(r_un[:], tmp[:], axis=mybir.AxisListType.X)
                nc.gpsimd.tensor_scalar(wT_sb[:, r:r + 1], r_un[:], recip[:],
                                        seecbc[:, BPC + r:BPC + r + 1],
                                        OP.mult, OP.mult)

            # -------------------------------------------- misc block into main
            ps_main = ps.tile([128, BPC], F32, tag="main")
            for t in range(3):
                nc.tensor.matmul(ps_main[:],
                                 w0m[:, t * 128:(t + 1) * 128],
                                 xmt[:, t * BPC:(t + 1) * BPC],
                                 start=(t == 0), stop=False)

            # output bias row early: ps_out accumulates br3 now, x4 @ Wr3 later
            ps_out = ps.tile([BPC, 128], F32, tag="psout")
            nc.tensor.matmul(ps_out[:], onesrow[0:1, 0:BPC], br3row[:],
                             start=True, stop=False)

            # -------------------------------------------- G^T build (128 MMs)
            ps_gt = ps.tile([128, 128], F32, tag="gt")
            for i in range(128):
                ch, off = tile_view[i]
                nc.tensor.matmul(ps_gt[:, i:i + 1], ch[:, off:off + 128],
                                 whcol, start=True, stop=True)
            gt_sb = sb.tile([128, 128], BF16, tag="gtsb")
            nc.vector.tensor_copy(gt_sb[:], ps_gt[:])
            ps_g = ps.tile([128, 128], BF16, tag="g")
            nc.tensor.transpose(ps_g[:], gt_sb[:], ident)
            g_sb = sb.tile([128, 128], BF16, tag="gsb")
            nc.vector.tensor_copy(g_sb[:], ps_g[:])

            # -------------------------------------------- Hm block: w @ G
            nc.tensor.matmul(ps_main[:], g_sb[:], wT_sb[:],
                             start=False, stop=False)

            # -------------------------------------------- alpha block (128 MMs)
            for j in range(128):
                ch, off = tile_view[128 + j]
                nc.tensor.matmul(ps_main[:], ch[:, off:off + 128],
                                 alpha_all[:, j:BPC * N:128],
                                 start=False, stop=(j == 127))

            # -------------------------------------------- bias+relu + MLP
            # layers 0-2 transposed [128h, b]; layer 3 emits [b, 128h] directly
            # (lhsT = x4^T, bias pre-accumulated via ones-matmul) so the out
            # DMA is 2 contiguous rows. relu+bias fused on DVE (one op/hop).
            xT = sb.tile([128, BPC], BF16, tag="x1T")
            nc.vector.tensor_scalar(xT[:], ps_main[:], b0v, 0.0,
                                    OP.add, OP.max)
            for t in range(3):
                ps_l = ps.tile([128, BPC], F32, tag="psl")
                nc.tensor.matmul(ps_l[:], wrv[:, t * 128:(t + 1) * 128],
                                 xT[:], start=True, stop=True)
                xT_next = sb.tile([128, BPC], BF16, tag=f"x{t + 2}T")
                nc.vector.tensor_scalar(xT_next[:], ps_l[:], brT[:, t:t + 1],
                                        0.0, OP.add, OP.max)
                xT = xT_next
            nc.tensor.matmul(ps_out[:], xT[:], wrv[:, 384:512],
                             start=False, stop=True)
            out_sb = sb.tile([BPC, 128], F32, tag="outsb")
            nc.vector.tensor_scalar(out_sb[:], ps_out[:], 0.0, None, OP.max)
            nc.sync.dma_start(t_out, out_sb[:])
    nc.compile()
    return nc


_compiled_nc = None


def get_nc():
    global _compiled_nc
    if _compiled_nc is None:
        _compiled_nc = build_nc()
    return _compiled_nc


def gather(results):
    """[2, 128] per core -> [16, 128] full output (pure unshard)."""
    return np.concatenate(
        [np.asarray(results[c]["out"], dtype=np.float32)
         for c in range(NCORES)], axis=0)


def kernel(**inputs):
    nc = get_nc()
    in_maps = shard_inputs(**inputs)
    res = bass_utils.run_bass_kernel_spmd(nc, in_maps, core_ids=list(range(NCORES)))
    return gather(res.results)


if __name__ == "__main__":
    nc = build_nc()
    print("build + compile OK;", len(nc.main_func.blocks), "blocks")
